# revision 14
# baseline (speedup 1.0000x reference)
"""CrossAttention GTrXL kernel for 8 Trainium2 NeuronCores.

Sharding: tensor-parallel over heads. 16 heads / 8 cores = 2 heads per core.

Core design (v2): the kernel is ScalarE(exp)-bound: 16.8M exp elems at
1 elem/cycle/lane @1.2GHz ~= 147us. Everything else hides behind it:
  - attention windows of 512 s-cols; per t-tile ONE [128, 1024] activation
    covers both heads (full-N ScalarE efficiency), psc double-buffered
    (2x2 PSUM banks) so ScalarE never waits for scores.
  - av matmuls use a [128t, 128] stationary: 64 val cols + 64 ones cols.
    M=128 keeps FWL on (no serial LDWEIGHTS) and rows 64..127 of the
    accumulator all hold the softmax denominator -> free broadcast for
    the normalize (reciprocal on [64, 512], no partition_broadcast).
  - PSUM: psc 4 banks + avp 2 + filler pool 2 = exactly 8.
  - 8 small AllGathers (1MB out each) spread evenly; outproj(w) runs as
    filler inside window w+2; tail exposes only the last AG.
  - projections are emitted as credit-paced filler units inside the
    attention loops; scores are emitted one iteration ahead.
"""

from collections import deque

import numpy as np
import ml_dtypes

import concourse.mybir as mybir
import concourse.tile as tile
from concourse import bacc
from concourse.bass_utils import run_bass_kernel_spmd

BF16 = mybir.dt.bfloat16
F32 = mybir.dt.float32
F8 = mybir.dt.float8e4

S = 2048   # dec seq
T = 2048   # enc seq
B = 2
E = 1024
H = 16
D = 64
HD = H * D           # 1024
N_CORES = 8
H_LOC = H // N_CORES  # 2 heads per core
PD = H_LOC * D        # 128 partition dims per core
BS = B * S            # 4096
BT = B * T            # 4096
SCALE = 1.0 / D ** 0.5

KT = E // 128        # 8 contraction tiles for projections
TTI = T // 128       # 16 t-tiles per batch
VBLK = 2 * 128       # val block per t-tile: 2 heads x (64 vals + 64 ones)
SWW = 512            # s window width
SW = S // SWW        # 4 windows per batch
NW = B * SW          # 8 windows total


def build_program():
    nc = bacc.Bacc("TRN2", target_bir_lowering=False, debug=False,
                   num_devices=N_CORES)

    # ---- I/O -----------------------------------------------------------
    xT = nc.dram_tensor("xT", [E, BS], BF16, kind="ExternalInput")       # inputs^T, col = b*S+s
    eT = nc.dram_tensor("eT", [E, BT], BF16, kind="ExternalInput")       # enc^T, col = b*T+t
    wq = nc.dram_tensor("wq", [E, PD], BF16, kind="ExternalInput")
    wk = nc.dram_tensor("wk", [E, PD], BF16, kind="ExternalInput")
    wv = nc.dram_tensor("wv", [E, PD], BF16, kind="ExternalInput")
    wp = nc.dram_tensor("wp", [HD, PD], BF16, kind="ExternalInput")      # full rows, my E-cols
    qbias = nc.dram_tensor("qbias", [PD, 1], F32, kind="ExternalInput")  # u + bq (per hd)
    kbias = nc.dram_tensor("kbias", [PD, 1], F32, kind="ExternalInput")  # bkv k-part
    bpcol = nc.dram_tensor("bpcol", [PD, 1], F32, kind="ExternalInput")  # bp slice col (+v-bias fold)
    out = nc.dram_tensor("out", [PD, BS], F32, kind="ExternalOutput")    # out^T: [my E cols, b*S+s]

    with tile.TileContext(nc) as tc:
        with tc.tile_pool(name="persist", bufs=1) as persist, \
             tc.tile_pool(name="dram", bufs=1, space="DRAM") as dram, \
             tc.tile_pool(name="psc", bufs=2, space="PSUM") as pscp, \
             tc.tile_pool(name="avp", bufs=2, space="PSUM") as avpp, \
             tc.tile_pool(name="fps", bufs=2, space="PSUM") as fps, \
             tc.tile_pool(name="att", bufs=4) as att, \
             tc.tile_pool(name="nrm", bufs=4) as nrm, \
             tc.tile_pool(name="agp", bufs=2) as agp, \
             tc.tile_pool(name="otp", bufs=2) as otp:

            # persistent SBUF tensors
            qT_b = [persist.tile([PD, S], BF16, tag=f"qT{b}", name=f"qT{b}") for b in range(B)]
            kT_b = [persist.tile([PD, T], BF16, tag=f"kT{b}", name=f"kT{b}") for b in range(B)]
            val_b = [persist.tile([128, TTI * VBLK], BF16, tag=f"val{b}", name=f"val{b}")
                     for b in range(B)]
            qb_sb = persist.tile([PD, 1], F32)
            kb_sb = persist.tile([PD, 1], F32)
            bp_sb = persist.tile([PD, 1], F32)
            wq_sb = persist.tile([128, KT * PD], BF16)  # k-tiles side by side
            wk_sb = persist.tile([128, KT * PD], BF16)
            wv_sb = persist.tile([128, KT * PD], BF16)
            wp_sb = persist.tile([128, KT * PD], BF16)

            for b in range(B):
                nc.vector.memset(val_b[b][:], 1.0)  # ones cols survive copies
            # weights + biases: consolidated DMAs on the scalar queue
            # (ScalarE is idle until the first activation)
            nc.scalar.dma_start(out=qb_sb[:], in_=qbias.ap())
            nc.scalar.dma_start(out=kb_sb[:], in_=kbias.ap())
            nc.scalar.dma_start(out=bp_sb[:], in_=bpcol.ap())
            for w_sb, w_dr in ((wq_sb, wq), (wk_sb, wk), (wv_sb, wv),
                               (wp_sb, wp)):
                nc.scalar.dma_start(
                    out=w_sb.rearrange("p (k d) -> p k d", k=KT),
                    in_=w_dr.rearrange("(k p) d -> p k d", p=128))

            # input tiles. b0 arrives split (first 512 cols first, so the
            # first window can start early); b1 reuses the same SBUF tags
            # (WAR deps free b0's buffers once its projections are done).
            eT_t = {}
            xT_t = {}
            for k in range(KT):
                et = persist.tile([128, T], BF16, tag=f"et{k}", name=f"et0_{k}")
                nc.gpsimd.dma_start(out=et[:, 0:SWW],
                                    in_=eT[k * 128:(k + 1) * 128, 0:SWW])
                eT_t[(0, k)] = et
                xt = persist.tile([128, S], BF16, tag=f"xt{k}", name=f"xt0_{k}")
                nc.sync.dma_start(out=xt[:, 0:SWW],
                                  in_=xT[k * 128:(k + 1) * 128, 0:SWW])
                xT_t[(0, k)] = xt
            for k in range(KT):
                nc.sync.dma_start(out=eT_t[(0, k)][:, SWW:2 * SWW],
                                  in_=eT[k * 128:(k + 1) * 128, SWW:2 * SWW])
            for k in range(KT):
                nc.sync.dma_start(out=eT_t[(0, k)][:, 2 * SWW:T],
                                  in_=eT[k * 128:(k + 1) * 128, 2 * SWW:T])
            for k in range(KT):
                nc.sync.dma_start(out=xT_t[(0, k)][:, SWW:S],
                                  in_=xT[k * 128:(k + 1) * 128, SWW:S])

            def load_b1_inputs():
                for k in range(KT):
                    et = persist.tile([128, T], BF16, tag=f"et{k}",
                                      name=f"et1_{k}")
                    nc.sync.dma_start(out=et[:, 0:SWW],
                                      in_=eT[k * 128:(k + 1) * 128,
                                             T:T + SWW])
                    eT_t[(1, k)] = et
                for k in range(KT):
                    nc.sync.dma_start(out=eT_t[(1, k)][:, SWW:T],
                                      in_=eT[k * 128:(k + 1) * 128,
                                             T + SWW:2 * T])
                for k in range(KT):
                    xt = persist.tile([128, S], BF16, tag=f"xt{k}",
                                      name=f"xt1_{k}")
                    nc.gpsimd.dma_start(out=xt[:],
                                        in_=xT[k * 128:(k + 1) * 128, S:2 * S])
                    xT_t[(1, k)] = xt

            # DRAM bounce + AllGather buffers, one per window
            windows = [(b, sw) for b in range(B) for sw in range(SW)]
            av_dram = {w: dram.tile([PD, SWW], BF16, tag=f"avd{w}", name=f"avd{w}")
                       for w in windows}
            ag_dram = {w: dram.tile([HD, SWW], BF16, tag=f"agd{w}", name=f"agd{w}",
                                    addr_space="Shared")
                       for w in windows}

            # ---------------- filler unit generators --------------------
            # each returns (cost_in_pe_ns_estimate, emit_fn)
            def proj_unit(b, ch, which):
                """512-wide chunk of the qT / kT projection for batch b."""
                def emit():
                    src = xT_t if which == "q" else eT_t
                    w = wq_sb if which == "q" else wk_sb
                    bias = qb_sb if which == "q" else kb_sb
                    dst = qT_b[b] if which == "q" else kT_b[b]
                    p = fps.tile([PD, 512], F32, tag="fp", name="pproj")
                    for k in range(KT):
                        nc.tensor.matmul(p[:], w[:, k * PD:(k + 1) * PD],
                                         src[(b, k)][:, ch * 512:(ch + 1) * 512],
                                         start=(k == 0), stop=(k == KT - 1))
                    nc.vector.tensor_scalar_add(
                        dst[:, ch * 512:(ch + 1) * 512], p[:], bias[:])
                return (1900, emit)

            def val_unit(b, ti):
                """One 128-row t-tile of the val projection for batch b."""
                def emit():
                    p = fps.tile([128, PD], F32, tag="fp", name="pval")
                    for k in range(KT):
                        nc.tensor.matmul(p[:],
                                         eT_t[(b, k)][:, ti * 128:(ti + 1) * 128],
                                         wv_sb[:, k * PD:(k + 1) * PD],
                                         start=(k == 0), stop=(k == KT - 1))
                    for h in range(H_LOC):
                        nc.vector.tensor_copy(
                            val_b[b][:, ti * VBLK + h * 128:
                                     ti * VBLK + h * 128 + 64],
                            p[:, h * 64:(h + 1) * 64])
                return (900, emit)

            def op_load_unit(w):
                """Load the AllGathered av for window w (one big DMA)."""
                holder = {}

                def emit():
                    a = agp.tile([128, KT * SWW], BF16, tag="ag", name="ag")
                    b, sw = w
                    nc.scalar.dma_start(
                        out=a.rearrange("p (k s) -> p k s", k=KT),
                        in_=ag_dram[w].rearrange("(k p) s -> p k s", p=128))
                    holder["t"] = a
                return (80, emit), holder

            def op_mm_unit(w, holder):
                """Output projection for window w from the loaded ag tile."""
                def emit():
                    b, sw = w
                    a = holder["t"]
                    po = fps.tile([128, SWW], F32, tag="fp", name="po")
                    for k in range(KT):
                        nc.tensor.matmul(
                            po[:], wp_sb[:, k * PD:(k + 1) * PD],
                            a[:, k * SWW:(k + 1) * SWW],
                            start=(k == 0), stop=(k == KT - 1))
                    o = otp.tile([128, SWW], F32, tag="o", name="o")
                    nc.vector.tensor_scalar_add(o[:], po[:], bp_sb[:])
                    col = b * S + sw * SWW
                    nc.sync.dma_start(out=out[:, col:col + SWW], in_=o[:])
                return (1900, emit)

            def dma_unit(fn):
                def emit():
                    fn()
                return (0, emit)

            # ---------------- attention for one window ------------------
            def attention_window(b, sw, filler, credit_per_iter=520):
                """One 512-col s-window of attention for batch b.

                Emission is software-pipelined: scores(ti+1) are emitted
                before act(ti) consumers so ScalarE never waits. Filler
                units are popped on a PE-cycle credit budget.
                """
                scol = sw * SWW
                avh = [avpp.tile([128, SWW], F32, tag="av", name=f"avh{h}")
                       for h in range(H_LOC)]
                credit = 0.0

                def scores(ti):
                    psc = pscp.tile([128, 2 * SWW], F32, tag="psc", name="psc")
                    for h in range(H_LOC):
                        nc.tensor.matmul(
                            psc[:, h * SWW:(h + 1) * SWW],
                            kT_b[b][h * 64:(h + 1) * 64, ti * 128:(ti + 1) * 128],
                            qT_b[b][h * 64:(h + 1) * 64, scol:scol + SWW],
                            start=True, stop=True)
                    return psc

                psc_cur = scores(0)
                for ti in range(TTI):
                    psc_next = scores(ti + 1) if ti + 1 < TTI else None
                    p_sb = att.tile([128, 2 * SWW], BF16, tag="p", name="p_sb")
                    nc.scalar.activation(
                        p_sb[:], psc_cur[:],
                        mybir.ActivationFunctionType.Exp, scale=SCALE)
                    credit += credit_per_iter
                    while filler and credit >= filler[0][0]:
                        cost, emit = filler.popleft()
                        credit -= cost
                        emit()
                    for h in range(H_LOC):
                        nc.tensor.matmul(
                            avh[h][:],
                            val_b[b][:, ti * VBLK + h * 128:
                                     ti * VBLK + (h + 1) * 128],
                            p_sb[:, h * SWW:(h + 1) * SWW],
                            start=(ti == 0), stop=(ti == TTI - 1))
                    psc_cur = psc_next

                # normalize + evict: rows 64..127 of avh all hold sum(exp)
                av_st = nrm.tile([D, 2 * SWW], BF16, tag="avst", name="av_st")
                for h in range(H_LOC):
                    z = nrm.tile([D, SWW], F32, tag="z", name="z")
                    nc.vector.tensor_copy(z[:], avh[h][64:128, :])
                    zi = nrm.tile([D, SWW], F32, tag="zi", name="zi")
                    nc.vector.reciprocal_approx_fast(zi[:], z[:])
                    nc.vector.tensor_mul(
                        av_st[:, h * SWW:(h + 1) * SWW],
                        avh[h][0:64, :], zi[:])
                w = (b, sw)
                nc.sync.dma_start(
                    out=av_dram[w].rearrange("(h d) s -> d h s", h=H_LOC),
                    in_=av_st.rearrange("d (h s) -> d h s", h=H_LOC))
                nc.gpsimd.collective_compute(
                    "AllGather", mybir.AluOpType.bypass,
                    replica_groups=[list(range(N_CORES))],
                    ins=[av_dram[w].opt()], outs=[ag_dram[w].opt()])

            # ---------------- schedule ---------------------------------
            # prefix: minimum to start window (b0, sw0)
            for _, emit in [proj_unit(0, 0, "q"), proj_unit(0, 0, "k"),
                            val_unit(0, 0), val_unit(0, 1), val_unit(0, 2)]:
                emit()

            def make_op(w):
                lu, holder = op_load_unit(w)
                return [lu, op_mm_unit(w, holder)]

            def run_window(b, sw, units, credit, must_empty=False):
                f = deque(units)
                attention_window(b, sw, f, credit_per_iter=credit)
                if must_empty:
                    assert not f, f"window {(b, sw)} filler starved: " \
                                  f"{len(f)} units left"
                for _, emit in f:
                    emit()

            # window 0: rest of b0's t-side, deadline-ordered (val(ti) is
            # consumed at iter ti, k-chunk c from iter 4c; q(0,1) by w1)
            run_window(0, 0, [
                val_unit(0, 3), val_unit(0, 4), proj_unit(0, 1, "k"),
                val_unit(0, 5), val_unit(0, 6), proj_unit(0, 2, "k"),
                val_unit(0, 7), val_unit(0, 8), val_unit(0, 9),
                val_unit(0, 10), proj_unit(0, 3, "k"), val_unit(0, 11),
                val_unit(0, 12), val_unit(0, 13), val_unit(0, 14),
                val_unit(0, 15), proj_unit(0, 1, "q")],
                credit=1300, must_empty=True)

            # b0 windows 1-3: finish b0 q chunks, stream b1 inputs, start
            # b1 projections; outproj(w) runs ~2 windows after w's AG
            run_window(0, 1, [
                proj_unit(0, 2, "q"), proj_unit(0, 3, "q"),
                dma_unit(load_b1_inputs)], credit=1000, must_empty=True)
            run_window(0, 2, [
                proj_unit(1, 0, "k"), val_unit(1, 0), val_unit(1, 1),
                val_unit(1, 2)], credit=520, must_empty=True)
            run_window(0, 3, [
                proj_unit(1, 1, "k"), val_unit(1, 3), val_unit(1, 4),
                val_unit(1, 5), val_unit(1, 6), val_unit(1, 7),
                val_unit(1, 8), proj_unit(1, 0, "q")] + make_op((0, 0)),
                credit=700, must_empty=True)

            # b1 windows
            run_window(1, 0, [
                proj_unit(1, 2, "k"), val_unit(1, 9), val_unit(1, 10),
                proj_unit(1, 1, "q"), val_unit(1, 11), proj_unit(1, 3, "k"),
                val_unit(1, 12), val_unit(1, 13), val_unit(1, 14),
                val_unit(1, 15)] + make_op((0, 1)),
                credit=880, must_empty=True)
            run_window(1, 1, [proj_unit(1, 2, "q")] + make_op((0, 2))
                       + make_op((0, 3)), credit=520)
            run_window(1, 2, [proj_unit(1, 3, "q")] + make_op((1, 0)),
                       credit=520)
            run_window(1, 3, make_op((1, 1)), credit=520)

            # tail: outproj for the last two windows; keep PE warm with
            # projection replays while the final AGs land
            for _, emit in make_op((1, 2)):
                emit()
            last = make_op((1, 3))
            last[0][1]()            # arm the ag load
            for i in range(4):
                proj_unit(1, i, "q")[1]()
            last[1][1]()

    nc.compile()
    return nc


_NC_CACHE = None


def _get_program():
    global _NC_CACHE
    if _NC_CACHE is None:
        _NC_CACHE = build_program()
    return _NC_CACHE


def _make_in_maps(inputs, pos_embedding, encoder_hidden_states, u, v, mask,
                  Wkv, bkv, Wq, bq, Wp, bp):
    bf = ml_dtypes.bfloat16
    xT = np.ascontiguousarray(
        np.asarray(inputs, np.float32).transpose(2, 1, 0).reshape(E, BS)).astype(bf)
    eT = np.ascontiguousarray(
        np.asarray(encoder_hidden_states, np.float32).transpose(2, 1, 0)
        .reshape(E, BT)).astype(bf)
    Wkv = np.asarray(Wkv, np.float32)
    Wq = np.asarray(Wq, np.float32)
    Wp = np.asarray(Wp, np.float32)
    bkv = np.asarray(bkv, np.float32)
    bq = np.asarray(bq, np.float32)
    bp = np.asarray(bp, np.float32)
    uf = np.asarray(u, np.float32).reshape(HD)
    in_maps = []
    for c in range(N_CORES):
        sl = slice(c * PD, (c + 1) * PD)
        in_maps.append({
            "xT": xT,
            "eT": eT,
            "wq": np.ascontiguousarray(Wq[:, sl]).astype(bf),
            "wk": np.ascontiguousarray(Wkv[:, sl]).astype(bf),
            "wv": np.ascontiguousarray(Wkv[:, HD + c * PD: HD + (c + 1) * PD]).astype(bf),
            "wp": np.ascontiguousarray(Wp[:, sl]).astype(bf),
            "qbias": (uf[sl] + bq[sl]).reshape(PD, 1).astype(np.float32),
            "kbias": bkv[sl].reshape(PD, 1).astype(np.float32),
            "bpcol": (bp[sl] + bkv[HD:] @ Wp[:, sl]).reshape(PD, 1)
                     .astype(np.float32),
        })
    return in_maps


def _assemble(results):
    full = np.empty((S, B, E), np.float32)
    for c in range(N_CORES):
        part = np.asarray(results[c]["out"]).reshape(PD, B, S)
        full[:, :, c * PD:(c + 1) * PD] = part.transpose(2, 1, 0)
    return full


def run(trace=False, **inputs):
    nc = _get_program()
    in_maps = _make_in_maps(**inputs)
    res = run_bass_kernel_spmd(nc, in_maps, core_ids=list(range(N_CORES)),
                               trace=trace)
    return _assemble(res.results), res


def kernel(**inputs):
    out, _ = run(**inputs)
    return out
